# revision 9
# baseline (speedup 1.0000x reference)
"""Trainium2 Bass kernel for top-2-of-8 MoE (T=4096, H=1024, I=1024).

Strategy (sparse routed grouped-GEMM, expert-sharded, 8 cores):
  - Routing (softmax + top-2 + renormalize) is computed on the HOST from the
    router logits (T x 8 — trivial), giving per-pair (token, expert, weight).
  - Each core owns exactly ONE expert: its full up/down weights (6 MB bf16)
    plus only the tokens routed to it (~1024 of 8192 pairs), padded to a
    compile-time capacity C (multiple of 128).
  - Device dataflow is transpose-free:
      up:   hT[i_chunk, pairs] = Wup[h, i_chunk].T @ xT[h, pairs]
            (weights stationary, token columns streamed; output is h
             TRANSPOSED with I on partitions — exactly what down needs)
      act:  h = silu(gate) * up     (ACT Silu + DVE multiply)
      down: y[pair_tile, H] = hT[:, pair_tile].T @ Wdn[i, H]
            (PSUM-accumulated over the 8 I-chunks)
      scale: y *= combine_weight (per-partition scalar on ACT) -> DMA out f32
  - No collectives: each pair's full down-projection lives on one core.
    The host gathers per-core pair rows and adds the two pairs per token.

Compute dtype bf16 (f32 PSUM accumulation), f32 output.
"""

import os
import sys

for _p in ("/opt/trn_rl_repo",):
    if _p not in sys.path:
        sys.path.append(_p)

import numpy as np
import ml_dtypes

import concourse.bass as bass
import concourse.bacc as bacc
import concourse.mybir as mybir
import concourse.tile as tile
from concourse.bass_utils import run_bass_kernel_spmd

BF16 = mybir.dt.bfloat16
F32 = mybir.dt.float32
AX = mybir.AxisListType
OP = mybir.AluOpType
AF = mybir.ActivationFunctionType

N_CORES = 8
H = 1024
I_FULL = 1024
E = 8
K_TOP = 2
KT = H // 128  # 8 contraction k-tiles for the up GEMM
IC = I_FULL // 128  # 8 I-chunks
P = 128


def _rearrange(x, pattern, **kw):
    import einops

    return np.ascontiguousarray(einops.rearrange(x, pattern, **kw))


def _chunks(C):
    out = []
    c0 = 0
    while c0 < C:
        cw = min(512, C - c0)
        out.append((c0, cw))
        c0 += cw
    return out


def build_graph(C):
    """SPMD graph: one expert per core, capacity C pairs (multiple of 128)."""
    NTI = C // P  # pair tiles
    chunks = _chunks(C)

    nc = bacc.Bacc("TRN2", target_bir_lowering=False, debug=False,
                   num_devices=N_CORES)

    xt_ext = nc.dram_tensor("xt", [P, KT * C], BF16, kind="ExternalInput")
    wup_ext = nc.dram_tensor("wup", [P, IC * 2048], BF16, kind="ExternalInput")
    wd_ext = nc.dram_tensor("wd", [P, IC * H], BF16, kind="ExternalInput")
    wsc_ext = nc.dram_tensor("wsc", [P, NTI], F32, kind="ExternalInput")
    out_ext = nc.dram_tensor("out", [C, H], BF16, kind="ExternalOutput")

    with tile.TileContext(nc) as tc:
        with (
            tc.tile_pool(name="big", bufs=1) as big,
            tc.tile_pool(name="work", bufs=2) as work,
            tc.tile_pool(name="hbuf", bufs=1) as hbuf,
            tc.tile_pool(name="outp", bufs=2) as outp,
            tc.tile_pool(name="pup", bufs=1, space="PSUM") as pup,
            tc.tile_pool(name="pdn", bufs=1, space="PSUM") as pdn,
        ):
            xt = big.tile([P, KT * C], BF16)
            wup = big.tile([P, IC * 2048], BF16)
            wd = big.tile([P, IC * H], BF16)
            wsc = big.tile([P, NTI], F32)

            # Batched DMAs issued from the (otherwise idle) GpSimd queue, in
            # first-use order on the PE. Each DMA_DIRECT2D issue costs
            # ~0.6us serially on its queue, so fewer+bigger beats many+small.
            def xt_chunk_dma(c0, cw):
                dst = xt[:].rearrange("p (k c) -> p k c", k=KT)[:, :, c0:c0 + cw]
                src = xt_ext[:].rearrange("p (k c) -> p k c", k=KT)[:, :, c0:c0 + cw]
                nc.gpsimd.dma_start(dst, src)

            nc.gpsimd.dma_start(wsc[:], wsc_ext[:])
            xt_chunk_dma(*chunks[0])
            for ip0 in range(0, IC, 2):
                nc.gpsimd.dma_start(wup[:, ip0 * 2048:(ip0 + 2) * 2048],
                                    wup_ext[:, ip0 * 2048:(ip0 + 2) * 2048])
            if len(chunks) > 1:
                xt_chunk_dma(*chunks[1])
            nc.gpsimd.dma_start(wd[:], wd_ext[:])
            for (c0, cw) in chunks[2:]:
                xt_chunk_dma(c0, cw)

            hT = {}

            def up_chunk(cc):
                c0, cw = chunks[cc]
                gen = cc % 2
                for ip in range(IC):
                    pgu = pup.tile([P, 1024], F32, tag="pgu%d" % (ip % 2),
                                   name="pgu_%d_%d" % (cc, ip))
                    pg = pgu[:, 0:512]
                    pu = pgu[:, 512:1024]
                    for k in range(KT):
                        w0 = ip * 2048 + k * 256
                        nc.tensor.matmul(
                            pg[:, :cw], wup[:, w0: w0 + 128],
                            xt[:, k * C + c0: k * C + c0 + cw],
                            start=(k == 0), stop=(k == KT - 1))
                    for k in range(KT):
                        w0 = ip * 2048 + k * 256 + 128
                        nc.tensor.matmul(
                            pu[:, :cw], wup[:, w0: w0 + 128],
                            xt[:, k * C + c0: k * C + c0 + cw],
                            start=(k == 0), stop=(k == KT - 1))
                    sg = work.tile([P, 512], F32, tag="sg")
                    nc.scalar.activation(sg[:, :cw], pg[:, :cw], AF.Silu)
                    ht = hbuf.tile([P, 512], BF16, tag="h%d_%d" % (gen, ip),
                                   name="h_%d_%d" % (cc, ip))
                    nc.vector.tensor_tensor(ht[:, :cw], sg[:, :cw],
                                            pu[:, :cw], op=OP.mult)
                    hT[(gen, ip)] = ht

            def down_chunk(cc):
                c0, cw = chunks[cc]
                gen = cc % 2
                for tt in range(cw // P):
                    gt = c0 // P + tt
                    y = pdn.tile([P, H], F32, tag="y%d" % (tt % 2),
                                 name="y_%d" % gt)
                    for ip in range(IC):
                        lhs = hT[(gen, ip)][:, tt * P: (tt + 1) * P]
                        nc.tensor.matmul(y[:, 0:512], lhs,
                                         wd[:, ip * H: ip * H + 512],
                                         start=(ip == 0), stop=(ip == IC - 1))
                        nc.tensor.matmul(y[:, 512:H], lhs,
                                         wd[:, ip * H + 512: (ip + 1) * H],
                                         start=(ip == 0), stop=(ip == IC - 1))
                    ysb = outp.tile([P, H], BF16, tag="ysb")
                    nc.scalar.mul(ysb[:], y[:], wsc[:, gt: gt + 1])
                    nc.sync.dma_start(out_ext[gt * P:(gt + 1) * P, :], ysb[:])

            # software pipeline: down(cc-1) is emitted after up(cc) so the PE
            # queue never stalls waiting for the activation of chunk cc.
            for cc in range(len(chunks)):
                up_chunk(cc)
                if cc > 0:
                    down_chunk(cc - 1)
            down_chunk(len(chunks) - 1)

    nc.compile()
    return nc


def route(router_logits):
    """Host top-2 routing, bit-matching the reference's top_k semantics."""
    T = router_logits.shape[0]
    m = router_logits.max(-1, keepdims=True)
    ex = np.exp(router_logits - m)
    p = ex / ex.sum(-1, keepdims=True)
    rows = np.arange(T)
    a1 = np.argmax(p, axis=-1)
    p1 = p[rows, a1]
    pm = p.copy()
    pm[rows, a1] = -1.0
    a2 = np.argmax(pm, axis=-1)
    p2 = p[rows, a2]
    s = p1 + p2
    return a1, a2, p1 / s, p2 / s


def make_in_maps(hidden_states, router_logits, up_weight, down_weight):
    """Host routing + per-core (per-expert) input prep.

    Returns (in_maps, pos, C): pos[t, slot] is the row in the concatenated
    [8*C, H] device output holding that pair's (already weighted) result.
    """
    T = hidden_states.shape[0]
    bf = ml_dtypes.bfloat16
    a1, a2, w1, w2 = route(router_logits.astype(np.float32))
    counts = np.bincount(a1, minlength=E) + np.bincount(a2, minlength=E)
    C = max(1152, int(-(-counts.max() // P) * P))

    x16 = hidden_states.astype(bf)
    pos = np.empty((T, 2), dtype=np.int64)
    in_maps = []
    for e in range(E):
        t1 = np.flatnonzero(a1 == e)
        t2 = np.flatnonzero(a2 == e)
        pos[t1, 0] = e * C + np.arange(len(t1))
        pos[t2, 1] = e * C + len(t1) + np.arange(len(t2))
        cnt = len(t1) + len(t2)

        xpad = np.zeros((C, H), dtype=bf)
        xpad[:len(t1)] = x16[t1]
        xpad[len(t1):cnt] = x16[t2]
        xt = _rearrange(xpad, "c (k p) -> p (k c)", p=P)

        wpad = np.zeros((C,), dtype=np.float32)
        wpad[:len(t1)] = w1[t1]
        wpad[len(t1):cnt] = w2[t2]
        wsc = _rearrange(wpad, "(t p) -> p t", p=P)

        W = up_weight[e].astype(bf)
        Wg = W[:, :I_FULL].reshape(KT, P, IC, P)
        Wu = W[:, I_FULL:].reshape(KT, P, IC, P)
        wup = _rearrange(np.stack([Wg, Wu], axis=3), "k p i s q -> p (i k s q)")

        wdn = _rearrange(down_weight[e].astype(bf), "(i p) h -> p (i h)", p=P)

        in_maps.append({"xt": xt, "wup": wup, "wd": wdn, "wsc": wsc})
    return in_maps, pos, C


_GRAPH_CACHE = {}


def _get_graph(C):
    if C not in _GRAPH_CACHE:
        _GRAPH_CACHE[C] = build_graph(C)
    return _GRAPH_CACHE[C]


def kernel(hidden_states, router_logits, up_weight, down_weight, topk,
           trace=False):
    assert int(topk) == K_TOP
    hidden_states = np.asarray(hidden_states, dtype=np.float32)
    router_logits = np.asarray(router_logits, dtype=np.float32)
    up_weight = np.asarray(up_weight, dtype=np.float32)
    down_weight = np.asarray(down_weight, dtype=np.float32)

    in_maps, pos, C = make_in_maps(hidden_states, router_logits,
                                   up_weight, down_weight)
    nc = _get_graph(C)
    res = run_bass_kernel_spmd(nc, in_maps, list(range(N_CORES)), trace=trace)
    Y = np.concatenate([res.results[r]["out"].astype(np.float32)
                        for r in range(N_CORES)], axis=0)
    out = Y[pos[:, 0]] + Y[pos[:, 1]]
    kernel.last_exec_time_ns = res.exec_time_ns
    return out


kernel.last_exec_time_ns = None


# revision 12
# speedup vs baseline: 1.1734x; 1.1734x over previous
"""Trainium2 Bass kernel for top-2-of-8 MoE (T=4096, H=1024, I=1024).

Strategy (sparse routed grouped-GEMM, expert-sharded, 8 cores):
  - Routing (softmax + top-2 + renormalize) is computed on the HOST from the
    router logits (T x 8 — trivial), giving per-pair (token, expert, weight).
  - Each core owns exactly ONE expert: its full up/down weights (6 MB bf16)
    plus only the tokens routed to it (~1024 of 8192 pairs), padded to a
    compile-time capacity C (multiple of 128).
  - Device dataflow is transpose-free:
      up:   hT[i_chunk, pairs] = Wup[h, i_chunk].T @ xT[h, pairs]
            (weights stationary, token columns streamed; output is h
             TRANSPOSED with I on partitions — exactly what down needs)
      act:  h = silu(gate) * up     (ACT Silu + DVE multiply)
      down: y[pair_tile, H] = hT[:, pair_tile].T @ Wdn[i, H]
            (PSUM-accumulated over the 8 I-chunks)
      scale: y *= combine_weight (per-partition scalar on ACT) -> DMA out f32
  - No collectives: each pair's full down-projection lives on one core.
    The host gathers per-core pair rows and adds the two pairs per token.

Compute dtype bf16 (f32 PSUM accumulation), f32 output.
"""

import os
import sys

for _p in ("/opt/trn_rl_repo",):
    if _p not in sys.path:
        sys.path.append(_p)

import numpy as np
import ml_dtypes

import concourse.bass as bass
import concourse.bacc as bacc
import concourse.mybir as mybir
import concourse.tile as tile
from concourse.bass_utils import run_bass_kernel_spmd

BF16 = mybir.dt.bfloat16
F32 = mybir.dt.float32
AX = mybir.AxisListType
OP = mybir.AluOpType
AF = mybir.ActivationFunctionType

N_CORES = 8
H = 1024
I_FULL = 1024
E = 8
K_TOP = 2
KT = H // 128  # 8 contraction k-tiles for the up GEMM
IC = I_FULL // 128  # 8 I-chunks
P = 128


def _rearrange(x, pattern, **kw):
    import einops

    return np.ascontiguousarray(einops.rearrange(x, pattern, **kw))


def _chunks(C):
    out = []
    c0 = 0
    while c0 < C:
        cw = min(512, C - c0)
        out.append((c0, cw))
        c0 += cw
    return out


def build_graph(C):
    """SPMD graph: one expert per core, capacity C pairs (multiple of 128)."""
    NTI = C // P  # pair tiles
    chunks = _chunks(C)

    nc = bacc.Bacc("TRN2", target_bir_lowering=False, debug=False,
                   num_devices=N_CORES)

    xt_ext = nc.dram_tensor("xt", [P, KT * C], BF16, kind="ExternalInput")
    wup_ext = nc.dram_tensor("wup", [P, IC * 2048], BF16, kind="ExternalInput")
    wd_ext = nc.dram_tensor("wd", [P, IC * H], BF16, kind="ExternalInput")
    wsc_ext = nc.dram_tensor("wsc", [P, NTI], F32, kind="ExternalInput")
    out_ext = nc.dram_tensor("out", [C, H], BF16, kind="ExternalOutput")

    with tile.TileContext(nc) as tc:
        with (
            tc.tile_pool(name="big", bufs=1) as big,
            tc.tile_pool(name="work", bufs=2) as work,
            tc.tile_pool(name="hbuf", bufs=1) as hbuf,
            tc.tile_pool(name="outp", bufs=2) as outp,
            tc.tile_pool(name="pup", bufs=1, space="PSUM") as pup,
            tc.tile_pool(name="pdn", bufs=1, space="PSUM") as pdn,
        ):
            xt = big.tile([P, KT * C], BF16)
            wup = big.tile([P, IC * 2048], BF16)
            wd = big.tile([P, IC * H], BF16)
            wsc = big.tile([P, NTI], F32)

            # DMA issue costs ~0.6us serially per queue; only SP (sync),
            # Activation (scalar) and gpsimd can issue, and gpsimd is slow.
            # Split: chunk-0 tokens on sync (first thing it does), up weights
            # on scalar (parallel with sync), and everything needed later
            # (chunk-1/2 tokens, down weights) drip-fed from the scalar queue
            # between silu ops via the `prefetch` list consumed in up_chunk.
            nc.sync.dma_start(wsc[:], wsc_ext[:])
            c0, cw = chunks[0]
            for k in range(KT):
                nc.sync.dma_start(xt[:, k * C + c0: k * C + c0 + cw],
                                  xt_ext[:, k * C + c0: k * C + c0 + cw])
            for ip in range(IC):
                nc.scalar.dma_start(wup[:, ip * 2048:(ip + 1) * 2048],
                                    wup_ext[:, ip * 2048:(ip + 1) * 2048])

            prefetch = []
            for (c0, cw) in chunks[1:2]:
                for k in range(KT):
                    prefetch.append((xt[:, k * C + c0: k * C + c0 + cw],
                                     xt_ext[:, k * C + c0: k * C + c0 + cw]))
            for ip0 in range(0, IC, 2):
                prefetch.append((wd[:, ip0 * H:(ip0 + 2) * H],
                                 wd_ext[:, ip0 * H:(ip0 + 2) * H]))
            for (c0, cw) in chunks[2:]:
                for k in range(KT):
                    prefetch.append((xt[:, k * C + c0: k * C + c0 + cw],
                                     xt_ext[:, k * C + c0: k * C + c0 + cw]))

            hT = {}

            def up_chunk(cc):
                c0, cw = chunks[cc]
                gen = cc % 2
                for ip in range(IC):
                    pgu = pup.tile([P, 1024], F32, tag="pgu%d" % (ip % 2),
                                   name="pgu_%d_%d" % (cc, ip))
                    pg = pgu[:, 0:512]
                    pu = pgu[:, 512:1024]
                    for k in range(KT):
                        w0 = ip * 2048 + k * 256
                        nc.tensor.matmul(
                            pg[:, :cw], wup[:, w0: w0 + 128],
                            xt[:, k * C + c0: k * C + c0 + cw],
                            start=(k == 0), stop=(k == KT - 1))
                    for k in range(KT):
                        w0 = ip * 2048 + k * 256 + 128
                        nc.tensor.matmul(
                            pu[:, :cw], wup[:, w0: w0 + 128],
                            xt[:, k * C + c0: k * C + c0 + cw],
                            start=(k == 0), stop=(k == KT - 1))
                    sg = work.tile([P, 512], F32, tag="sg")
                    nc.scalar.activation(sg[:, :cw], pg[:, :cw], AF.Silu)
                    for _ in range(3):
                        if prefetch:
                            dst, src = prefetch.pop(0)
                            nc.scalar.dma_start(dst, src)
                    ht = hbuf.tile([P, 512], BF16, tag="h%d_%d" % (gen, ip),
                                   name="h_%d_%d" % (cc, ip))
                    nc.vector.tensor_tensor(ht[:, :cw], sg[:, :cw],
                                            pu[:, :cw], op=OP.mult)
                    hT[(gen, ip)] = ht

            def down_chunk(cc):
                c0, cw = chunks[cc]
                gen = cc % 2
                for tt in range(cw // P):
                    gt = c0 // P + tt
                    y = pdn.tile([P, H], F32, tag="y%d" % (tt % 2),
                                 name="y_%d" % gt)
                    for ip in range(IC):
                        lhs = hT[(gen, ip)][:, tt * P: (tt + 1) * P]
                        nc.tensor.matmul(y[:, 0:512], lhs,
                                         wd[:, ip * H: ip * H + 512],
                                         start=(ip == 0), stop=(ip == IC - 1))
                        nc.tensor.matmul(y[:, 512:H], lhs,
                                         wd[:, ip * H + 512: (ip + 1) * H],
                                         start=(ip == 0), stop=(ip == IC - 1))
                    ysb = outp.tile([P, H], BF16, tag="ysb")
                    nc.scalar.mul(ysb[:], y[:], wsc[:, gt: gt + 1])
                    nc.sync.dma_start(out_ext[gt * P:(gt + 1) * P, :], ysb[:])

            # software pipeline: down(cc-1) is emitted after up(cc) so the PE
            # queue never stalls waiting for the activation of chunk cc.
            for cc in range(len(chunks)):
                up_chunk(cc)
                if cc > 0:
                    down_chunk(cc - 1)
            down_chunk(len(chunks) - 1)

    nc.compile()
    return nc


def route(router_logits):
    """Host top-2 routing, bit-matching the reference's top_k semantics."""
    T = router_logits.shape[0]
    m = router_logits.max(-1, keepdims=True)
    ex = np.exp(router_logits - m)
    p = ex / ex.sum(-1, keepdims=True)
    rows = np.arange(T)
    a1 = np.argmax(p, axis=-1)
    p1 = p[rows, a1]
    pm = p.copy()
    pm[rows, a1] = -1.0
    a2 = np.argmax(pm, axis=-1)
    p2 = p[rows, a2]
    s = p1 + p2
    return a1, a2, p1 / s, p2 / s


def make_in_maps(hidden_states, router_logits, up_weight, down_weight):
    """Host routing + per-core (per-expert) input prep.

    Returns (in_maps, pos, C): pos[t, slot] is the row in the concatenated
    [8*C, H] device output holding that pair's (already weighted) result.
    """
    T = hidden_states.shape[0]
    bf = ml_dtypes.bfloat16
    a1, a2, w1, w2 = route(router_logits.astype(np.float32))
    counts = np.bincount(a1, minlength=E) + np.bincount(a2, minlength=E)
    C = max(1152, int(-(-counts.max() // P) * P))

    x16 = hidden_states.astype(bf)
    pos = np.empty((T, 2), dtype=np.int64)
    in_maps = []
    for e in range(E):
        t1 = np.flatnonzero(a1 == e)
        t2 = np.flatnonzero(a2 == e)
        pos[t1, 0] = e * C + np.arange(len(t1))
        pos[t2, 1] = e * C + len(t1) + np.arange(len(t2))
        cnt = len(t1) + len(t2)

        xpad = np.zeros((C, H), dtype=bf)
        xpad[:len(t1)] = x16[t1]
        xpad[len(t1):cnt] = x16[t2]
        xt = _rearrange(xpad, "c (k p) -> p (k c)", p=P)

        wpad = np.zeros((C,), dtype=np.float32)
        wpad[:len(t1)] = w1[t1]
        wpad[len(t1):cnt] = w2[t2]
        wsc = _rearrange(wpad, "(t p) -> p t", p=P)

        W = up_weight[e].astype(bf)
        Wg = W[:, :I_FULL].reshape(KT, P, IC, P)
        Wu = W[:, I_FULL:].reshape(KT, P, IC, P)
        wup = _rearrange(np.stack([Wg, Wu], axis=3), "k p i s q -> p (i k s q)")

        wdn = _rearrange(down_weight[e].astype(bf), "(i p) h -> p (i h)", p=P)

        in_maps.append({"xt": xt, "wup": wup, "wd": wdn, "wsc": wsc})
    return in_maps, pos, C


_GRAPH_CACHE = {}


def _get_graph(C):
    if C not in _GRAPH_CACHE:
        _GRAPH_CACHE[C] = build_graph(C)
    return _GRAPH_CACHE[C]


def kernel(hidden_states, router_logits, up_weight, down_weight, topk,
           trace=False):
    assert int(topk) == K_TOP
    hidden_states = np.asarray(hidden_states, dtype=np.float32)
    router_logits = np.asarray(router_logits, dtype=np.float32)
    up_weight = np.asarray(up_weight, dtype=np.float32)
    down_weight = np.asarray(down_weight, dtype=np.float32)

    in_maps, pos, C = make_in_maps(hidden_states, router_logits,
                                   up_weight, down_weight)
    nc = _get_graph(C)
    res = run_bass_kernel_spmd(nc, in_maps, list(range(N_CORES)), trace=trace)
    Y = np.concatenate([res.results[r]["out"].astype(np.float32)
                        for r in range(N_CORES)], axis=0)
    out = Y[pos[:, 0]] + Y[pos[:, 1]]
    kernel.last_exec_time_ns = res.exec_time_ns
    return out


kernel.last_exec_time_ns = None
